# revision 1
# baseline (speedup 1.0000x reference)
"""Trainium2 Bass kernel for the DataReloadingQNN problem.

Math: layers 0..4 of the circuit are sample-independent -> one shared state
v4. Layer 5 applies, per qubit q, shared B_q = RZ RY RZ then the data gate
RY(x_q) = cos(x_q/2) I + sin(x_q/2) J. Expanding the product over qubits,
    state_b = sum_m W[b, m] * u_m,   m in [0, 2048)
where W = tensor product of the per-qubit [cos, sin] pairs (bit q of m picks
cos/sin) and u_m are 2048 shared vectors derived from params only (CNOT
permutation folded in). Device work per core (1024 samples):
  1. cos/sin of x/2 on ScalarE
  2. W (128 x 2048 per sample-tile) by doubling with per-partition
     tensor_scalar multiplies on VectorE
  3. transpose W via PE, then S = W @ U as bf16 matmuls, K=2048, N=4096
  4. PSUM -> SBUF copy, DMA out f32
Inputs are sharded batch-wise across 8 cores; U (params-derived) replicated.
"""
import numpy as np
import ml_dtypes

import concourse.bass as bass
import concourse.bacc as bacc
import concourse.tile as tile
from concourse import mybir
from concourse.bass_utils import run_bass_kernel_spmd

N = 11
DIM = 2048
BATCH = 8192
NCORES = 8
BSH = BATCH // NCORES          # 1024 samples per core
NTILES = BSH // 128            # 8 sample-tiles per core
KT = DIM // 128                # 16 k-tiles
NCH = 8                        # output column chunks
NW = (2 * DIM) // NCH          # 512 columns per chunk
F32 = mybir.dt.float32
BF16 = mybir.dt.bfloat16

# ---------------------------------------------------------------- host math

def _rz(phi):
    e = np.exp(-0.5j * phi)
    return np.array([[e, 0], [0, np.conj(e)]], dtype=np.complex128)


def _ry(theta):
    t = 0.5 * theta
    c, s = np.cos(t), np.sin(t)
    return np.array([[c, -s], [s, c]], dtype=np.complex128)


def _apply_1q_rows(rows, U, q):
    R = rows.shape[0]
    st = rows.reshape(R, 2 ** q, 2, 2 ** (N - 1 - q))
    st = np.einsum('ab,rxby->rxay', U, st)
    return st.reshape(R, DIM)


def _apply_cnot_rows(rows, c):
    R = rows.shape[0]
    st = rows.reshape(R, 2 ** c, 2, 2, 2 ** (N - 2 - c))
    st = np.stack([st[:, :, 0], st[:, :, 1, ::-1]], axis=2)
    return st.reshape(R, DIM)


def build_u_matrix(params):
    """(6,11,3) f32 -> U (2048, 4096) f64, re/im interleaved columns."""
    p = params.astype(np.float64)
    v = np.zeros((1, DIM), dtype=np.complex128)
    v[0, 0] = 1.0
    for l in range(5):
        for q in range(N):
            v = _apply_1q_rows(v, _rz(p[l, q, 0]), q)
            v = _apply_1q_rows(v, _ry(p[l, q, 1]), q)
            v = _apply_1q_rows(v, _rz(p[l, q, 2]), q)
        for c in range(N - 1):
            v = _apply_cnot_rows(v, c)

    J = np.array([[0, -1], [1, 0]], dtype=np.complex128)
    rows = v
    for q in range(N):
        B = _rz(p[5, q, 2]) @ _ry(p[5, q, 1]) @ _rz(p[5, q, 0])
        rb = _apply_1q_rows(rows, B, q)
        rc = _apply_1q_rows(rows, J @ B, q)
        rows = np.concatenate([rb, rc], axis=0)

    g = np.arange(DIM)[None, :]
    for c in range(N - 1):
        g = _apply_cnot_rows(g.astype(np.float64), c).astype(np.int64)
    rows = rows[:, g[0]]

    U = np.empty((DIM, 2 * DIM), dtype=np.float64)
    U[:, 0::2] = rows.real
    U[:, 1::2] = rows.imag
    return U

# ------------------------------------------------------------- bass kernel

def build_kernel():
    nc = bacc.Bacc()
    x_d = nc.dram_tensor("x", (BSH, N), F32, kind="ExternalInput")
    u_d = nc.dram_tensor("u", (KT, 128, 2 * DIM), BF16, kind="ExternalInput")
    id_d = nc.dram_tensor("ident", (128, 128), BF16, kind="ExternalInput")
    out_d = nc.dram_tensor("out", (BSH, 2 * DIM), F32, kind="ExternalOutput")

    with tile.TileContext(nc) as tc:
        with (
            tc.tile_pool(name="const", bufs=1) as const_pool,
            tc.tile_pool(name="wbuild", bufs=2) as wbuild_pool,
            tc.tile_pool(name="wt", bufs=1) as wt_pool,
            tc.tile_pool(name="uin", bufs=2) as u_pool,
            tc.tile_pool(name="outs", bufs=6) as out_pool,
            tc.tile_pool(name="ptr", bufs=2, space=bass.MemorySpace.PSUM) as ptr_pool,
            tc.tile_pool(name="pmm", bufs=4, space=bass.MemorySpace.PSUM) as pmm_pool,
        ):
            ident = const_pool.tile([128, 128], BF16)
            nc.gpsimd.dma_start(ident[:], id_d[:])

            # x: (1024, 11) -> sbuf (128, 8*11) in one DMA (3D access pattern);
            # sample-tile t lives in cols [t*11, (t+1)*11)
            x_sb = const_pool.tile([128, NTILES * N], F32)
            x_r = x_d.rearrange("(t p) f -> p t f", p=128)
            nc.gpsimd.dma_start(x_sb[:].rearrange("p (t f) -> p t f", f=N), x_r)

            cos_sb = const_pool.tile([128, NTILES * N], F32)
            sin_sb = const_pool.tile([128, NTILES * N], F32)
            hp_t = const_pool.tile([128, 1], F32)
            zr_t = const_pool.tile([128, 1], F32)
            nc.vector.memset(hp_t[:], float(np.pi / 2))
            nc.vector.memset(zr_t[:], 0.0)
            # cos(t) = sin(pi/2 - t): keeps Sin args in (-pi/2, pi/2], the
            # ACT table is inaccurate beyond pi
            nc.scalar.activation(cos_sb[:], x_sb[:],
                                 mybir.ActivationFunctionType.Sin,
                                 bias=hp_t[:], scale=-0.5)
            nc.scalar.activation(sin_sb[:], x_sb[:],
                                 mybir.ActivationFunctionType.Sin,
                                 bias=zr_t[:], scale=0.5)

            # Phase A: build transposed W for every sample-tile
            wts = []
            for t in range(NTILES):
                col = t * N
                wa = wbuild_pool.tile([128, DIM], F32, tag="wa")
                wb = wbuild_pool.tile([128, DIM], F32, tag="wb")
                nc.vector.tensor_copy(wa[:, 0:1], cos_sb[:, col:col + 1])
                nc.vector.tensor_copy(wa[:, 1:2], sin_sb[:, col:col + 1])
                cur, nxt = wa, wb
                for j in range(1, N):
                    half = 1 << j
                    nc.vector.tensor_scalar_mul(
                        nxt[:, 0:half], cur[:, 0:half],
                        cos_sb[:, col + j:col + j + 1])
                    nc.vector.tensor_scalar_mul(
                        nxt[:, half:2 * half], cur[:, 0:half],
                        sin_sb[:, col + j:col + j + 1])
                    cur, nxt = nxt, cur
                # cast to bf16 for the PE
                wbf = wbuild_pool.tile([128, DIM], BF16, tag="wbf")
                nc.vector.tensor_copy(wbf[:], cur[:])

                wt = wt_pool.tile([128, KT * 128], BF16, tag=f"wt{t}")
                for k in range(KT):
                    ptr = ptr_pool.tile([128, 128], BF16)
                    nc.tensor.transpose(ptr[:], wbf[:, k * 128:(k + 1) * 128],
                                        ident[:])
                    nc.vector.tensor_copy(wt[:, k * 128:(k + 1) * 128], ptr[:])
                wts.append(wt)

            # Phase B: stream U by column chunk, matmul all sample-tiles
            for ci in range(NCH):
                ut = u_pool.tile([128, KT * NW], BF16, tag="u")
                for k in range(KT):
                    nc.sync.dma_start(ut[:, k * NW:(k + 1) * NW],
                                      u_d[k, :, ci * NW:(ci + 1) * NW])
                for t in range(NTILES):
                    pmm = pmm_pool.tile([128, NW], F32)
                    for k in range(KT):
                        nc.tensor.matmul(
                            pmm[:],
                            wts[t][:, k * 128:(k + 1) * 128],
                            ut[:, k * NW:(k + 1) * NW],
                            start=(k == 0), stop=(k == KT - 1))
                    ot = out_pool.tile([128, NW], F32)
                    nc.scalar.copy(ot[:], pmm[:])
                    nc.sync.dma_start(
                        out_d[t * 128:(t + 1) * 128, ci * NW:(ci + 1) * NW],
                        ot[:])
    nc.finalize()
    return nc

# ----------------------------------------------------------------- driver

_CACHE = {}


def kernel(X, params):
    X = np.ascontiguousarray(np.asarray(X, dtype=np.float32))
    params = np.asarray(params, dtype=np.float32)

    U = build_u_matrix(params)
    u_bf = np.ascontiguousarray(
        U.reshape(KT, 128, 2 * DIM).astype(ml_dtypes.bfloat16))
    ident = np.eye(128, dtype=ml_dtypes.bfloat16)

    if "nc" not in _CACHE:
        _CACHE["nc"] = build_kernel()
    nc = _CACHE["nc"]

    in_maps = []
    for c in range(NCORES):
        in_maps.append({
            "x": X[c * BSH:(c + 1) * BSH],
            "u": u_bf,
            "ident": ident,
        })
    res = run_bass_kernel_spmd(nc, in_maps, list(range(NCORES)))
    out = np.concatenate([res.results[c]["out"] for c in range(NCORES)], axis=0)
    return out.reshape(BATCH, DIM, 2)

